# revision 25
# baseline (speedup 1.0000x reference)
"""BitLinear MLP (per-token int8 act fake-quant, per-tensor ternary weight
fake-quant, tanh-gelu) on 8 Trainium2 NeuronCores.

Sharding: data-parallel over tokens (B*S = 16384 -> 2048 tokens/core), weights
replicated. Weights are fake-quantized host-side to ternary bf16 plus an fp32
inverse scale; activations are quantized on-device to integer-valued bf16.
The quantized matmuls then accumulate exactly in fp32 PSUM (every partial sum
is an integer < 2^24), so device results match the reference up to the scale
divisions and the gelu LUT.

Per-core pipeline (all bf16 matmuls, fp32 PSUM):
  phase 0: load x tile [128, D], row absmax -> scale, round to int bf16
           (magic-number round-to-nearest-even; the [-128,127] clip never
           binds because |x*s| <= 127 by construction), PE-transpose into
           resident xqT [D, T] (SBUF).
  phase 1: h = gelu((xqT.T @ w1T) * inv_scale) streamed over w1 column chunks;
           gelu'd fp32 h rows stored to DRAM scratch; running row absmax kept.
  phase 2: quantize h rows, PE-transpose to hqT, out = (hqT.T @ w2T) * inv,
           written straight to the output.
"""

import sys

sys.path.insert(0, "/opt/trn_rl_repo")

from contextlib import ExitStack

import ml_dtypes
import numpy as np

import concourse.bass as bass
from concourse import bacc
import concourse.mybir as mybir
import concourse.tile as tile
from concourse.alu_op_type import AluOpType as ALU
from concourse.bass_utils import run_bass_kernel_spmd
from concourse.masks import make_identity

F32 = mybir.dt.float32
BF16 = mybir.dt.bfloat16
AXX = mybir.AxisListType.X
GELU_TANH = mybir.ActivationFunctionType.Gelu_apprx_tanh
F16 = mybir.dt.float16

B, S, D, H = 4, 4096, 2048, 8192
T = B * S
NCORES = 8
TPC = T // NCORES  # tokens per core
EPS = 1e-5
MAGIC = float(np.float32(1.5 * 2**23))  # add/sub -> round-to-nearest-even
P = 128


def build_nc(tpc: int, d: int, h: int) -> bass.Bass:
    assert tpc % P == 0 and d % 512 == 0 and h % 512 == 0
    NT = tpc // P  # token tiles
    KD = d // P  # layer-1 contraction blocks
    KH = h // P  # layer-2 contraction blocks
    NI = d // 512  # 512-wide output column tiles (layer 2)
    W1C = min(512, h)  # streamed w1T column chunk (double-buffered)
    assert h % W1C == 0
    NQ = h // W1C  # w1 chunk passes
    JT = W1C // 512  # 512-wide j tiles per chunk (psum group)
    assert JT <= 4
    TG = min(8, NT)  # phase-2 token-tile group (resident hqT)
    assert NT % TG == 0
    TB = 4  # transposes batched per psum->sbuf copy

    nc = bacc.Bacc(trn_type="TRN2")
    x = nc.dram_tensor("x", [tpc, d], F32, kind="ExternalInput")[:]
    w1t = nc.dram_tensor("w1t", [d, h], BF16, kind="ExternalInput")[:]
    w2t = nc.dram_tensor("w2t", [h, d], BF16, kind="ExternalInput")[:]
    wsc = nc.dram_tensor("wsc", [1, 2], F32, kind="ExternalInput")[:]
    out = nc.dram_tensor("out", [tpc, d], F32, kind="ExternalOutput")[:]

    with tile.TileContext(nc) as tc, ExitStack() as ctx:
        const = ctx.enter_context(tc.tile_pool(name="const", bufs=1))
        scl = ctx.enter_context(tc.tile_pool(name="scl", bufs=1))
        mmps = ctx.enter_context(tc.tile_pool(name="mmps", bufs=8, space="PSUM"))
        dram = ctx.enter_context(tc.tile_pool(name="dram", bufs=1, space="DRAM"))

        wsc_sb = const.tile([P, 2], F32)
        nc.gpsimd.dma_start(out=wsc_sb, in_=wsc.to_broadcast((P, 2)))
        magic_sb = const.tile([P, 1], F32)
        nc.vector.memset(magic_sb, MAGIC)

        # per-token scale state, one column per token tile
        xinv = scl.tile([P, NT], F32)  # (1/s_x) * (1/s_w1)
        hmax = scl.tile([P, NT], F32)  # running absmax of gelu(h) rows
        hscale = scl.tile([P, NT], F32)  # 127 / clip(hmax, EPS)
        hinv = scl.tile([P, NT], F32)  # (1/s_h) * (1/s_w2)
        nc.vector.memset(hmax, 0.0)

        hbuf = dram.tile([tpc, h], F16)
        hbufs = [hbuf[tt * P : (tt + 1) * P, :] for tt in range(NT)]

        with (
            tc.tile_pool(name="xqt", bufs=1) as xqt_pool,
            tc.tile_pool(name="w1sb", bufs=2) as w1_pool,
            tc.tile_pool(name="p1stage", bufs=2) as p1s,
            tc.tile_pool(name="p1small", bufs=4) as p1small,
        ):
            xqT = xqt_pool.tile([P, KD, NT, P], BF16)

            # ---- phase 0: quantize + transpose x ----
            for tt in range(NT):
                xt = p1s.tile([P, d], F32, tag="xt")
                nc.sync.dma_start(out=xt, in_=x[tt * P : (tt + 1) * P, :])
                xm = p1small.tile([P, 1], F32, tag="xm")
                nc.vector.reduce_max(xm, xt, axis=AXX, apply_absolute_value=True)
                nc.vector.tensor_scalar_max(xm, xm, EPS)
                xr = p1small.tile([P, 1], F32, tag="xr")
                nc.vector.reciprocal(xr, xm)
                xs = p1small.tile([P, 1], F32, tag="xs")
                nc.vector.tensor_scalar(xs, xr, 127.0, None, op0=ALU.mult)
                xi = p1small.tile([P, 1], F32, tag="xi")
                nc.vector.reciprocal(xi, xs)
                nc.vector.tensor_tensor(
                    xinv[:, tt : tt + 1], xi, wsc_sb[:, 0:1], op=ALU.mult
                )
                # round(x*s) to integer-valued bf16 (clip never binds)
                nc.scalar.activation(
                    xt, xt, mybir.ActivationFunctionType.Identity,
                    bias=magic_sb, scale=xs,
                )
                xq = p1s.tile([P, d], BF16, tag="xq")
                nc.vector.tensor_scalar(xq, xt, MAGIC, None, op0=ALU.subtract)
                nc.scalar.dma_start(out=xqT[:, :, tt, :], in_=xq, transpose=True)

            # ---- phase 1: h = gelu(xq @ w1q.T), store rows, track absmax ----
            for q in range(NQ):
                w1sb = w1_pool.tile([P, KD, W1C], BF16, tag="w1sb")
                for k4 in range(0, KD, 4):
                    nc.sync.dma_start(
                        out=w1sb[:, k4 : k4 + 4, :],
                        in_=w1t[
                            k4 * P : (k4 + 4) * P, q * W1C : (q + 1) * W1C
                        ].rearrange("(kk p) c -> p kk c", p=P),
                    )
                for tt in range(NT):
                    pss = [
                        mmps.tile([P, 512], F32, tag="mm", name=f"ps1_{j}")
                        for j in range(JT)
                    ]
                    for k in range(KD):
                        lhs = xqT[:, k, tt, :]
                        for j in range(JT):
                            nc.tensor.matmul(
                                pss[j],
                                lhsT=lhs,
                                rhs=w1sb[:, k, j * 512 : (j + 1) * 512],
                                start=(k == 0),
                                stop=(k == KD - 1),
                            )
                    hrow = p1s.tile([P, W1C], F16, tag="hrow")
                    for j in range(JT):
                        nc.scalar.activation(
                            hrow[:, j * 512 : (j + 1) * 512],
                            pss[j],
                            GELU_TANH,
                            scale=xinv[:, tt : tt + 1],
                        )
                    hm = p1small.tile([P, 1], F32, tag="hm")
                    nc.vector.reduce_max(
                        hm, hrow, axis=AXX, apply_absolute_value=True
                    )
                    nc.vector.tensor_tensor(
                        hmax[:, tt : tt + 1], hmax[:, tt : tt + 1], hm, op=ALU.max
                    )
                    nc.scalar.dma_start(
                        out=hbufs[tt][:, q * W1C : (q + 1) * W1C], in_=hrow
                    )
                    if q == NQ - 1:
                        # per-token-tile h scale, ready as soon as its row is
                        hs = hscale[:, tt : tt + 1]
                        nc.vector.tensor_scalar_max(
                            hs, hmax[:, tt : tt + 1], EPS
                        )
                        nc.vector.reciprocal(hs, hs)
                        nc.vector.tensor_scalar(hs, hs, 127.0, None, op0=ALU.mult)
                        hi = hinv[:, tt : tt + 1]
                        nc.vector.reciprocal(hi, hs)
                        nc.vector.tensor_tensor(
                            hi, hi, wsc_sb[:, 1:2], op=ALU.mult
                        )

        # ---- phase 2: quantize h, transpose, out = hq @ w2q.T ----
        HHALF = h // 4
        KHH = KH // 4
        with (
            tc.tile_pool(name="hqt", bufs=TG) as hqt_pool,
            tc.tile_pool(name="p2stage", bufs=2) as p2s,
            tc.tile_pool(name="p2g", bufs=2) as p2g,
            tc.tile_pool(name="p2q", bufs=3) as p2q,
            tc.tile_pool(name="w2sb", bufs=3) as w2_pool,
            tc.tile_pool(name="ostage", bufs=2) as op_pool,
        ):
            for g in range(NT // TG):
                hqTs = []
                for u in range(TG):
                    tt = g * TG + u
                    hqT = hqt_pool.tile([P, KH, P], BF16, tag="hqT")
                    for half in range(4):
                        hf = p2s.tile([P, HHALF], F16, tag="hf")
                        nc.sync.dma_start(
                            out=hf,
                            in_=hbufs[tt][:, half * HHALF : (half + 1) * HHALF],
                        )
                        hg = p2g.tile([P, HHALF], F32, tag="hg")
                        nc.scalar.activation(
                            hg, hf, mybir.ActivationFunctionType.Identity,
                            bias=magic_sb, scale=hscale[:, tt : tt + 1],
                        )
                        hq = p2q.tile([P, HHALF], BF16, tag="hq")
                        nc.vector.tensor_scalar(
                            hq, hg, MAGIC, None, op0=ALU.subtract
                        )
                        nc.scalar.dma_start(
                            out=hqT[:, half * KHH : (half + 1) * KHH, :],
                            in_=hq,
                            transpose=True,
                        )
                    hqTs.append(hqT)
                for iq in range(NI):
                    pss = [
                        mmps.tile([P, 512], F32, tag="mm", name=f"ps2_{u}")
                        for u in range(TG)
                    ]
                    for k4 in range(0, KH, 8):
                        w2sb = w2_pool.tile([P, 8, 512], BF16, tag="w2")
                        nc.sync.dma_start(
                            out=w2sb,
                            in_=w2t[
                                k4 * P : (k4 + 8) * P,
                                iq * 512 : (iq + 1) * 512,
                            ].rearrange("(kk p) c -> p kk c", p=P),
                        )
                        for kk in range(8):
                            k = k4 + kk
                            for u in range(TG):
                                nc.tensor.matmul(
                                    pss[u],
                                    lhsT=hqTs[u][:, k, :],
                                    rhs=w2sb[:, kk, :],
                                    start=(k == 0),
                                    stop=(k == KH - 1),
                                )
                    for u in range(TG):
                        tt = g * TG + u
                        ot = op_pool.tile([P, 512], F32, tag="ot")
                        if u % 2 == 0:
                            nc.scalar.activation(
                                ot, pss[u], mybir.ActivationFunctionType.Identity,
                                bias=0.0, scale=hinv[:, tt : tt + 1],
                            )
                        else:
                            nc.vector.tensor_scalar(
                                ot, pss[u], hinv[:, tt : tt + 1], None,
                                op0=ALU.mult,
                            )
                        nc.scalar.dma_start(
                            out=out[tt * P : (tt + 1) * P, iq * 512 : (iq + 1) * 512],
                            in_=ot,
                        )
    nc.compile()
    return nc


def _quant_weight_host(w: np.ndarray):
    """Mirror reference _weight_quant: ternary values + fp32 inverse scale."""
    w = np.asarray(w, dtype=np.float32)
    mean = np.maximum(np.mean(np.abs(w), dtype=np.float32), np.float32(EPS))
    scale = np.float32(1.0) / mean
    tern = np.clip(np.round(w * scale), np.float32(-1.0), np.float32(1.0))
    wT = np.ascontiguousarray(tern.T).astype(ml_dtypes.bfloat16)
    winv = np.float32(1.0) / scale
    return wT, winv


_built: dict = {}


def _get_nc(tpc, d, h):
    key = (tpc, d, h)
    if key not in _built:
        _built[key] = build_nc(*key)
    return _built[key]


def run(inputs, trace=False, shapes=None, ncores=NCORES):
    if shapes is None:
        b, s, d, h = B, S, D, H
    else:
        b, s, d, h = shapes
    t = b * s
    tpc = t // ncores
    x = np.ascontiguousarray(np.asarray(inputs["x"], np.float32).reshape(t, d))
    w1t, winv1 = _quant_weight_host(inputs["w1"])
    w2t, winv2 = _quant_weight_host(inputs["w2"])
    wsc = np.array([[winv1, winv2]], dtype=np.float32)
    in_maps = [
        {
            "x": np.ascontiguousarray(x[c * tpc : (c + 1) * tpc]),
            "w1t": w1t,
            "w2t": w2t,
            "wsc": wsc,
        }
        for c in range(ncores)
    ]
    nc = _get_nc(tpc, d, h)
    res = run_bass_kernel_spmd(
        nc, in_maps, core_ids=list(range(ncores)), trace=False
    )
    outf = np.concatenate([res.results[c]["out"] for c in range(ncores)], axis=0)
    return outf.reshape(b, s, d), res


def kernel(**inputs) -> np.ndarray:
    return run(inputs)[0]
